# revision 19
# baseline (speedup 1.0000x reference)
"""EnhancedLoRALinear Trainium2 kernel.

Computes, for x:[4,8192,1024] and torch-style weights (out,in):
    out = x @ (W + W_res)^T + b + sigmoid(x @ W_gate^T) * (2 * (x @ W_down^T) @ W_up^T)

Strategy:
  - Data-parallel: the 32768 tokens are split across 8 NeuronCores (4096 each);
    the small weight matrices are replicated.
  - Algebraic fold: main + residual share one matmul with Wc = W + W_res.
  - Precision split (rel-err budget 2e-2; ~1.7e-2 on HW, identical to the
    host-numpy emulation since inputs are deterministic):
      * main path: k<768 in bf16 (full PE rate, FWL weight loads); the
        k 768:1024 quarter in fp8 e4m3 perf_mode=DoubleRow (2 k-elements per
        cell -> 1 matmul instead of 2). The quarter's x is shipped as a
        separate tensor x8m = fp8(x/8) paired with wc8 = fp8(8*Wc) -- the
        product is unscaled (the /8 keeps 8*Wc's small values out of the
        fp8 subnormal range without needing a x64 psum rescale), so the DR
        matmul accumulates DIRECTLY into the main psum (start=False): no
        separate bank, no ACT rescale copy, and one less DVE add per half.
      * gate path entirely in fp8 DoubleRow: K=1024 in 4 matmuls. W_gate is
        scaled x64 to clear the fp8 subnormal range; the sigmoid applies
        scale=1/64. Sigmoid squashes the quantization error and the gate
        only multiplies the small LoRA term.
      * down-projection in fp8 DoubleRow (W_down scaled x64, zero-padded to
        128 output columns to keep a full-column LDWEIGHTS mask; the 1/64
        and the LoRA scaling 2.0 are folded into W_up host-side).
      * lora-up in bf16, zero-padded to K=128 (partial-row matmuls break
        LDWEIGHTS prefetching -- a row-tiled K=32 concurrent pair was tried
        and measured SLOWER: it stalls the DR-entry pipeline around it).
  - Per-tile PE order: bf16 mains, bf16 loras, then all fp8-DR work (main
    quarter into the open main psums, then gates) contiguously -- the DR
    weight-path entry is paid once per tile, adjacent to the next group's
    fp8 down-projection.
  - Epilogue per half: DVE adds bias to the main psum (freeing the bank),
    ACT sigmoid frees the gate psum, DVE mult (sig x lora) frees the lora
    psum, gpsimd does the all-SBUF final add. On the very last tile the
    h1 final runs on DVE and the output DMA is split per half across both
    queues to shorten the kernel tail.
  - DMA: two hwdge queues (Sync + Scalar), STRICTLY need-ordered in the
    prologue (big tensors use group-/half-major contiguous DRAM layouts:
    128 descriptors of 4-8KB; small descriptors throttle the queues; a
    4-queue prologue split was tried and measured slower -- HBM bandwidth
    is the early constraint and extra queues dilute the critical path's
    share). x prefetches two groups ahead mid-group on the Scalar queue.
    The very last tile reorders its PE work (lora/gates first, mains last)
    so the post-matmul chain is just bias-add -> final-add -> DMA.
  - Cold start: the junk-matmul HAM spin is the FIRST gpsimd+PE work (its
    memset precedes everything) so the PE clock gate opens during the DMA
    prologue. Group 0 runs as phase 1 (mains only, needing just wc + the
    bf16 x arriving in 0.25MB token-quarter DMAs) rotating over six PSUM
    banks with main+bias staged to SBUF; phase 2 (group 0's fp8 paths +
    combine) runs right BEFORE group 1 -- its epilogue overlaps group 1's
    matmuls and the kernel tail is just the last tile of group 7.
"""

import ml_dtypes
import numpy as np

_BF16 = ml_dtypes.bfloat16
_F8E4 = ml_dtypes.float8_e4m3  # IEEE e4m3 (bias 7, max 240) == TRN FP8_EXP4

import concourse.bass as bass
import concourse.bacc as bacc
import concourse.mybir as mybir
import concourse.tile as tile
from concourse.bass_utils import run_bass_kernel_spmd
from concourse.tile_rust import add_dep_helper

N_CORES = 8
B, S = 4, 8192
TOK = B * S                  # 32768 tokens total
T = TOK // N_CORES           # 4096 tokens per core
I = 1024                     # in_features
O = 1024                     # out_features
R = 16                       # lora rank
SCALING = 2.0                # lora_alpha / r
KT = I // 128                # 8 contraction tiles of 128
KB = 6                       # bf16 main k-tiles (k < 768); k 768:1024 in fp8
KJ = I // 256                # 4 fp8 DoubleRow contraction tiles
TG = 512                     # token group (down-projection batch)
NG = T // TG                 # 8 groups per core
NH = O // 512                # 2 output halves
WS = 64.0                    # fp8 weight scale (exact power of two)
WSM = 8.0                    # main-quarter split: x8m = fp8(x/8), wc8 = fp8(8w)
N_JUNK = 10                  # HAM warm-up matmuls (cover the DMA prologue)

F32 = mybir.dt.float32
BF16 = mybir.dt.bfloat16
F8 = mybir.dt.float8e4
DR = mybir.MatmulPerfMode.DoubleRow


def _build_nc():
    nc = bacc.Bacc(None)

    # Pre-swizzled DRAM layouts, group-/half-major so every DMA moves 128
    # contiguous 4-8KB rows (small descriptors throttle the DMA queues):
    #   bf16 x:   [g, 128p, 8k, TG]     k = kt*128 + p
    #   fp8  x:   [g, 128p, 4j, 2i, TG] k = j*256 + i*128 + p
    #   wc/wg:    [half, 128p, ..., 512]
    xtb = nc.dram_tensor("xtb", [NG, 128, KB, TG], BF16, kind="ExternalInput")
    # group-0 bf16 x again, tile-major, so phase-1's first chunk gates on a
    # 0.25MB quarter instead of the whole 1MB group
    xtb0q = nc.dram_tensor("xtb0q", [2, 128, KB, 256], BF16,
                           kind="ExternalInput")
    x8 = nc.dram_tensor("x8", [NG, 128, KJ, 2, TG], F8, kind="ExternalInput")
    wct = nc.dram_tensor("wct", [NH, 128, KB, 512], BF16, kind="ExternalInput")
    # last quarter of the main contraction (k 768:1024) as x8-scaled fp8,
    # paired with x8m = fp8(x/8) so the product lands unscaled in psum
    wc8d = nc.dram_tensor("wc8d", [NH, 128, 2, 512], F8, kind="ExternalInput")
    x8m = nc.dram_tensor("x8m", [NG, 128, 2, TG], F8, kind="ExternalInput")
    wg8 = nc.dram_tensor("wg8", [NH, 128, KJ, 2, 512], F8,
                         kind="ExternalInput")
    wd8 = nc.dram_tensor("wd8", [128, KJ, 2, 128], F8, kind="ExternalInput")
    wut2 = nc.dram_tensor("wut2", [128, O], BF16, kind="ExternalInput")
    biasr = nc.dram_tensor("biasr", [1, O], F32, kind="ExternalInput")
    out = nc.dram_tensor("out", [T, O], F32, kind="ExternalOutput")

    sig = mybir.ActivationFunctionType.Sigmoid
    mult = mybir.AluOpType.mult
    add = mybir.AluOpType.add

    with tile.TileContext(nc) as tc:
        with (
            tc.tile_pool(name="wpool", bufs=1) as wpool,
            tc.tile_pool(name="xpool", bufs=3) as xpool,
            tc.tile_pool(name="opool", bufs=3) as opool,
            tc.tile_pool(name="epool", bufs=3) as epool,
            tc.tile_pool(name="psum", bufs=1, space="PSUM") as pp,
        ):
            wc_h = [wpool.tile([128, KB, 512], BF16, name=f"wc{h}")
                    for h in range(NH)]
            wc8_h = [wpool.tile([128, 2, 512], F8, name=f"wc8{h}")
                     for h in range(NH)]
            wg_h = [wpool.tile([128, KJ, 2, 512], F8, name=f"wg{h}")
                    for h in range(NH)]
            wd_sb = wpool.tile([128, KJ, 2, 128], F8)
            # lora operands zero-padded to K=128 host-side (full-row matmuls
            # keep LDWEIGHTS prefetching)
            wu_sb = wpool.tile([128, O], BF16)
            down_pers = wpool.tile([128, TG], BF16)
            bias_r = wpool.tile([1, O], F32)
            bias_bc = wpool.tile([128, O], F32)

            x0_8 = xpool.tile([128, KJ, 2, TG], F8, tag="x8", name="x8_0")
            x0_8m = xpool.tile([128, 2, TG], F8, tag="x8m", name="x8m_0")
            x0_bq = [wpool.tile([128, KB, 256], BF16, name=f"xtb0q{c}")
                     for c in range(2)]

            # HAM spin-up: the junk memset is the FIRST gpsimd op so the PE
            # gets work as soon as the engines come up; the junk matmuls keep
            # it busy through the DMA prologue so the clock gate is open (and
            # stays open) before real compute starts
            junk = wpool.tile([128, 512], BF16)
            nc.gpsimd.memset(junk[:, :], 0.0)
            warm = pp.tile([128, 512], F32, tag="warm")
            spin = None
            for _ in range(N_JUNK):
                spin = nc.tensor.matmul(warm[:, :], junk[:, 0:128], junk[:, :],
                                        start=True, stop=True)
            # ordering-only deps: all junk matmuls precede the first matmul
            # of each psum bank's chain (collected as tags first appear)
            first_real = {}

            def note(tag, mm):
                if tag not in first_real:
                    first_real[tag] = mm

            # --- prologue DMAs on the two main queues, STRICTLY ordered by
            # first use: the first ~2.4MB (wc halves + bf16 x quarters) gate
            # phase 1, so nothing else may run ahead of them. (A 4-queue
            # split was tried and measured SLOWER: HBM bandwidth is the
            # constraint early on, and extra queues dilute the critical
            # path's share.)
            nc.gpsimd.memset(down_pers[:, :], 0.0)
            # sync: wc_h0 (phase-1 h0), half of wc_h1, then phase-2's fp8
            # tensors
            nc.sync.dma_start(out=wc_h[0][:, 0:3, :], in_=wct[0, :, 0:3, :])
            nc.sync.dma_start(out=wc_h[0][:, 3:6, :], in_=wct[0, :, 3:6, :])
            nc.sync.dma_start(out=wc_h[1][:, 0:3, :], in_=wct[1, :, 0:3, :])
            nc.sync.dma_start(out=x0_8[:, :, :, :], in_=x8[0, :, :, :, :])
            nc.sync.dma_start(out=wc8_h[0][:, :, :], in_=wc8d[0, :, :, :])
            nc.sync.dma_start(out=wc8_h[1][:, :, :], in_=wc8d[1, :, :, :])
            nc.sync.dma_start(out=wg_h[0][:, :, :, :], in_=wg8[0, :, :, :, :])
            # scalar: bias first (the phase-1 drains need it), the four bf16
            # x quarters for phase 1, the other half of wc_h1, phase-2's
            # small tensors, second wg half, group-1 x8
            nc.scalar.dma_start(out=bias_r[:, :], in_=biasr[:, :])
            for c in range(2):
                nc.scalar.dma_start(out=x0_bq[c][:, :, :],
                                    in_=xtb0q[c, :, :, :])
            nc.scalar.dma_start(out=wc_h[1][:, 3:6, :], in_=wct[1, :, 3:6, :])
            nc.scalar.dma_start(out=wd_sb[:, :, :, :], in_=wd8[:, :, :, :])
            nc.scalar.dma_start(out=x0_8m[:, :, :], in_=x8m[0, :, :, :])
            nc.scalar.dma_start(out=wu_sb[:, :], in_=wut2[:, :])
            nc.scalar.dma_start(out=wg_h[1][:, :, :, :], in_=wg8[1, :, :, :, :])
            x1_8 = xpool.tile([128, KJ, 2, TG], F8, tag="x8", name="x8_1")
            nc.scalar.dma_start(out=x1_8[:, :, :, :], in_=x8[1, :, :, :, :])
            nc.gpsimd.partition_broadcast(bias_bc[:, :], bias_r[0:1, :])

            # x-tile DMAs issue on the Scalar queue, one group ahead of use
            x_tiles = {}

            def issue_x(g, x8_t=None):
                if x8_t is None:
                    x8_t = xpool.tile([128, KJ, 2, TG], F8, tag="x8",
                                      name=f"x8_{g}")
                    nc.scalar.dma_start(
                        out=x8_t[:, :, :, :], in_=x8[g, :, :, :, :]
                    )
                xt_t = xpool.tile([128, KB, TG], BF16, tag="xtb",
                                  name=f"xtb{g}")
                nc.scalar.dma_start(
                    out=xt_t[:, :, :], in_=xtb[g, :, :, :]
                )
                xm_t = xpool.tile([128, 2, TG], F8, tag="x8m",
                                  name=f"x8m_{g}")
                nc.scalar.dma_start(
                    out=xm_t[:, :, :], in_=x8m[g, :, :, :]
                )
                x_tiles[g] = (xt_t, xm_t, x8_t)

            # --- phase 1: group-0 mains only (bf16 part; the fp8 quarter is
            # added in phase 2). Only wc and bf16-x are needed, so the PE
            # gets real work while the rest of the prologue is in flight.
            # main+bias results are staged in SBUF. ---
            P1_BANKS = ["main0", "main1", "gate0", "gate1", "lora0", "lora1"]
            g0mb = {}
            ci = 0
            for oh in range(NH):
                osl = slice(oh * 512, (oh + 1) * 512)
                for t in range(TG // 128):
                    mps = pp.tile([128, 512], F32, tag=P1_BANKS[ci % 6],
                                  name=f"p1ps{ci}")
                    for k in range(KB):
                        mm = nc.tensor.matmul(
                            mps[:, :],
                            x0_bq[t // 2][:, k, (t % 2) * 128 :
                                          (t % 2 + 1) * 128],
                            wc_h[oh][:, k, :],
                            start=(k == 0),
                            stop=(k == KB - 1),
                        )
                        if k == 0:
                            note(P1_BANKS[ci % 6], mm)
                    st_sb = wpool.tile([128, 512], F32, name=f"g0mb{t}_{oh}")
                    nc.vector.tensor_tensor(
                        st_sb[:, :], mps[:, :], bias_bc[:, osl], add
                    )
                    g0mb[(t, oh)] = st_sb
                    ci += 1

            issue_x(1, x8_t=x1_8)

            def do_down(g, x8_sb):
                dps = pp.tile([128, TG], F32, tag="dps", name=f"dps_g{g}")
                for j in range(KJ):
                    mm = nc.tensor.matmul(
                        dps[:, :],
                        wd_sb[:, j, :, :],
                        x8_sb[:, j, :, :],
                        start=(j == 0),
                        stop=(j == KJ - 1),
                        perf_mode=DR,
                    )
                    note("dps", mm)
                nc.vector.tensor_copy(down_pers[0:R, :], dps[0:R, :])

            def do_tile_tail(g, t, x8_sb, xm_sb, mset, quarter_done,
                             out_row0):
                """loras + fp8 paths (main quarter/gates) + combine for one
                128-token tile. mset[oh] is either an OPEN main psum tile
                (quarter_done=False: the DR quarter accumulates into it,
                then DVE adds bias) or a staged main+bias SBUF tile
                (quarter_done=True: phase 2, quarter via spare banks)."""
                tsl = slice(t * 128, (t + 1) * 128)
                out_sb = opool.tile([128, O], F32, tag="out",
                                    name=f"out_g{g}{t}")
                lps = {}
                for oh in range(NH):
                    osl = slice(oh * 512, (oh + 1) * 512)
                    lp_t = pp.tile([128, 512], F32, tag=f"lora{oh}",
                                   name=f"lp_g{g}{t}_{oh}")
                    lps[oh] = lp_t
                    mm = nc.tensor.matmul(
                        lp_t[:, :],
                        down_pers[:, tsl],
                        wu_sb[:, osl],
                        start=True,
                        stop=True,
                    )
                    note(f"lora{oh}", mm)
                # fp8-DR block: main quarter first, then gates -- one DR
                # weight-path entry for all of it
                fin = {}
                for oh in range(NH):
                    osl = slice(oh * 512, (oh + 1) * 512)
                    if not quarter_done:
                        # k 768:1024 accumulates into the still-open main
                        # psum; the product is unscaled (x/8 x 8w)
                        nc.tensor.matmul(
                            mset[oh][:, :],
                            xm_sb[:, :, tsl],
                            wc8_h[oh][:, :, :],
                            start=False,
                            stop=True,
                            perf_mode=DR,
                        )
                        mb_sb = epool.tile([128, 512], F32, tag=f"mb{oh}",
                                           name=f"mb_g{g}{t}_{oh}")
                        nc.vector.tensor_tensor(
                            mb_sb[:, :], mset[oh][:, :], bias_bc[:, osl], add
                        )
                        fin[oh] = mb_sb
                    else:
                        # phase 2: quarter into a spare bank, DVE pre-adds
                        # it with the staged main+bias
                        p8 = pp.tile([128, 512], F32,
                                     tag="warm" if oh == 0 else "dps",
                                     name=f"p8_{g}_{t}_{oh}")
                        mm = nc.tensor.matmul(
                            p8[:, :],
                            xm_sb[:, :, tsl],
                            wc8_h[oh][:, :, :],
                            start=True,
                            stop=True,
                            perf_mode=DR,
                        )
                        note("warm" if oh == 0 else "dps", mm)
                        pre_sb = epool.tile([128, 512], F32, tag=f"mb{oh}",
                                            name=f"pre_{g}{t}_{oh}")
                        nc.vector.tensor_tensor(
                            pre_sb[:, :], p8[:, :], mset[oh][:, :], add
                        )
                        fin[oh] = pre_sb
                gset = {}
                for oh in range(NH):
                    gps = pp.tile([128, 512], F32, tag=f"gate{oh}",
                                  name=f"gp_g{g}{t}_{oh}")
                    for j in range(KJ):
                        mm = nc.tensor.matmul(
                            gps[:, :],
                            x8_sb[:, j, :, tsl],
                            wg_h[oh][:, j, :, :],
                            start=(j == 0),
                            stop=(j == KJ - 1),
                            perf_mode=DR,
                        )
                        note(f"gate{oh}", mm)
                    gset[oh] = gps
                for oh in range(NH):
                    osl = slice(oh * 512, (oh + 1) * 512)
                    g_sb = epool.tile([128, 512], F32, tag=f"sig{oh}",
                                      name=f"sg_g{g}{t}_{oh}")
                    nc.scalar.activation(g_sb[:, :], gset[oh][:, :], sig,
                                         scale=1.0 / WS)
                    gl_sb = epool.tile([128, 512], F32, tag=f"gl{oh}",
                                       name=f"gg_g{g}{t}_{oh}")
                    nc.vector.tensor_tensor(
                        gl_sb[:, :], g_sb[:, :], lps[oh][:, :], mult
                    )
                    nc.gpsimd.tensor_tensor(
                        out_sb[:, osl], gl_sb[:, :], fin[oh][:, :], add
                    )
                nc.sync.dma_start(
                    out=out[out_row0 + t * 128 :
                            out_row0 + (t + 1) * 128, :],
                    in_=out_sb[:, :],
                )

            # --- phase 2: group 0's fp8 paths + combine, BEFORE group 1:
            # its epilogue overlaps group 1's matmuls instead of trailing
            # the whole kernel ---
            do_down(0, x0_8)
            for t in range(TG // 128):
                if t == 2:
                    issue_x(2)
                do_tile_tail(0, t, x0_8, x0_8m,
                             {oh: g0mb[(t, oh)] for oh in range(NH)},
                             True, 0)

            for g in range(1, NG):
                tg0 = g * TG
                xt_sb, xm_sb, x8_sb = x_tiles.pop(g)
                do_down(g, x8_sb)
                for t in range(TG // 128):
                    # prefetch the next-next group's x mid-group, after the
                    # DGE queue has drained (issuing early blocks the scalar
                    # engine on descriptor backpressure, delaying sigmoids)
                    if t == 2 and g + 2 < NG:
                        issue_x(g + 2)
                    if g == NG - 1 and t == TG // 128 - 1:
                        break  # very last tile handled below
                    tsl = slice(t * 128, (t + 1) * 128)
                    mset = {}
                    for oh in range(NH):
                        # double-buffer the main psums by tile parity (the
                        # warm/dps banks are free in steady state since the
                        # fp8 quarter accumulates in-place): tile t+1's
                        # mains never wait on tile t's DVE drain
                        mtag = (["main0", "main1"], ["warm", "dps"])[t % 2][oh]
                        mps = pp.tile([128, 512], F32, tag=mtag,
                                      name=f"mp_g{g}{t}_{oh}")
                        for k in range(KB):
                            mm = nc.tensor.matmul(
                                mps[:, :],
                                xt_sb[:, k, tsl],
                                wc_h[oh][:, k, :],
                                start=(k == 0),
                                stop=False,
                            )
                            note(f"main{oh}", mm)
                        mset[oh] = mps
                    do_tile_tail(g, t, x8_sb, xm_sb, mset, False, tg0)

            # --- very last tile, reordered: loras + gates + sigmoid + mult
            # run BEFORE the mains so the post-last-matmul chain is just
            # bias-add -> final-add -> DMA (per half, split across both
            # queues) ---
            g, t = NG - 1, TG // 128 - 1
            tg0 = g * TG
            tsl = slice(t * 128, (t + 1) * 128)
            out_sb = opool.tile([128, O], F32, tag="out", name="out_last")
            lps = {}
            for oh in range(NH):
                osl = slice(oh * 512, (oh + 1) * 512)
                lp_t = pp.tile([128, 512], F32, tag=f"lora{oh}",
                               name=f"lp_last{oh}")
                lps[oh] = lp_t
                nc.tensor.matmul(lp_t[:, :], down_pers[:, tsl],
                                 wu_sb[:, osl], start=True, stop=True)
            gl = {}
            for oh in range(NH):
                gps = pp.tile([128, 512], F32, tag=f"gate{oh}",
                              name=f"gp_last{oh}")
                for j in range(KJ):
                    nc.tensor.matmul(
                        gps[:, :], x8_sb[:, j, :, tsl], wg_h[oh][:, j, :, :],
                        start=(j == 0), stop=(j == KJ - 1), perf_mode=DR,
                    )
                g_sb = epool.tile([128, 512], F32, tag=f"sig{oh}",
                                  name=f"sg_last{oh}")
                nc.scalar.activation(g_sb[:, :], gps[:, :], sig,
                                     scale=1.0 / WS)
                gl_sb = epool.tile([128, 512], F32, tag=f"gl{oh}",
                                   name=f"gg_last{oh}")
                nc.vector.tensor_tensor(gl_sb[:, :], g_sb[:, :],
                                        lps[oh][:, :], mult)
                # fold the bias in early (off the critical path) so the
                # post-matmul chain is a single psum+sbuf add per half
                osl = slice(oh * 512, (oh + 1) * 512)
                bgl_sb = epool.tile([128, 512], F32, tag=f"mb{oh}",
                                    name=f"bgl_last{oh}")
                nc.gpsimd.tensor_tensor(bgl_sb[:, :], gl_sb[:, :],
                                        bias_bc[:, osl], add)
                gl[oh] = bgl_sb
            mset = {}
            for oh in range(NH):
                mps = pp.tile([128, 512], F32, tag=("warm", "dps")[oh],
                              name=f"mp_last{oh}")
                for k in range(KB):
                    nc.tensor.matmul(
                        mps[:, :], xt_sb[:, k, tsl], wc_h[oh][:, k, :],
                        start=(k == 0), stop=False,
                    )
                mset[oh] = mps
            for oh in range(NH):
                osl = slice(oh * 512, (oh + 1) * 512)
                nc.tensor.matmul(
                    mset[oh][:, :], xm_sb[:, :, tsl], wc8_h[oh][:, :, :],
                    start=False, stop=True, perf_mode=DR,
                )
                # DVE only: gpsimd cannot read PSUM
                nc.vector.tensor_tensor(out_sb[:, osl], mset[oh][:, :],
                                        gl[oh][:, :], add)
                q = nc.scalar if oh == 0 else nc.sync
                q.dma_start(
                    out=out[tg0 + t * 128 : tg0 + (t + 1) * 128, osl],
                    in_=out_sb[:, osl],
                )

            for fr in first_real.values():
                add_dep_helper(fr.ins, spin.ins, False,
                               "warmup before real matmuls")
    nc.compile()
    return nc


_NC_CACHE = None


def _get_nc():
    global _NC_CACHE
    if _NC_CACHE is None:
        _NC_CACHE = _build_nc()
    return _NC_CACHE


def _prep_inputs(x, W, b, W_down, W_up, W_gate, W_res):
    x = np.asarray(x, dtype=np.float32).reshape(TOK, I)
    # weights: [I, O] -> [half, 128p, kt(/pair), 512], k = kt*128 + p
    wcT64 = (np.asarray(W) + np.asarray(W_res)).T.astype(np.float32)
    wct = np.ascontiguousarray(
        wcT64[0 : KB * 128].astype(_BF16)
        .reshape(KB, 128, NH, 512).transpose(2, 1, 0, 3)
    )
    wc8d = np.ascontiguousarray(
        (WSM * wcT64[KB * 128 :]).astype(_F8E4)
        .reshape(2, 128, NH, 512).transpose(2, 1, 0, 3)
    )
    wgT = (WS * np.asarray(W_gate)).T.astype(_F8E4)
    wg8 = np.ascontiguousarray(
        wgT.reshape(KJ, 2, 128, NH, 512).transpose(3, 2, 0, 1, 4)
    )
    # wd zero-padded from R=16 to 128 output columns
    wdT = np.zeros((I, 128), dtype=_F8E4)
    wdT[:, 0:R] = (WS * np.asarray(W_down)).T.astype(_F8E4)
    wd8 = np.ascontiguousarray(
        wdT.reshape(KJ, 2, 128, 128).transpose(2, 0, 1, 3)
    )
    # lora-up weights: scaling/WS folded in; zero-padded to K=128 host-side
    wut2 = np.zeros((128, O), dtype=_BF16)
    wut2[0:R, :] = (SCALING / WS * np.asarray(W_up)).T.astype(_BF16)
    biasr = np.ascontiguousarray(np.asarray(b, dtype=np.float32).reshape(1, O))
    in_maps = []
    for c in range(N_CORES):
        xt_c = x[c * T : (c + 1) * T, :].T  # [I, T]
        # x: [I, T] -> [g, 128p, kt(/pair), TG], token t = g*TG + tau
        xtb_c = np.ascontiguousarray(
            xt_c[0 : KB * 128].astype(_BF16)
            .reshape(KB, 128, NG, TG).transpose(2, 1, 0, 3)
        )
        # tile-major copy of group 0 for the phase-1 quarter loads
        xtb0q_c = np.ascontiguousarray(
            xtb_c[0].reshape(128, KB, 2, 256).transpose(2, 0, 1, 3)
        )
        x8_c = np.ascontiguousarray(
            xt_c.astype(_F8E4).reshape(KJ, 2, 128, NG, TG)
            .transpose(3, 2, 0, 1, 4)
        )
        # main-quarter x, pre-divided by 8 (pairs with the 8x wc8d scale)
        x8m_c = np.ascontiguousarray(
            (xt_c[KB * 128 :] / np.float32(WSM)).astype(_F8E4)
            .reshape(2, 128, NG, TG).transpose(2, 1, 0, 3)
        )
        in_maps.append(
            {
                "xtb": xtb_c,
                "xtb0q": xtb0q_c,
                "wc8d": wc8d,
                "x8": x8_c,
                "x8m": x8m_c,
                "wct": wct,
                "wg8": wg8,
                "wd8": wd8,
                "wut2": wut2,
                "biasr": biasr,
            }
        )
    return in_maps


def run(inputs, trace=False, **kwargs):
    """Build + run on the 8 NeuronCores. Returns (full_output, BassKernelResults)."""
    nc = _get_nc()
    in_maps = _prep_inputs(**inputs)
    res = run_bass_kernel_spmd(
        nc, in_maps, list(range(N_CORES)), trace=trace, **kwargs
    )
    shards = [res.results[c]["out"] for c in range(N_CORES)]
    full = np.concatenate(shards, axis=0).reshape(B, S, O)
    return full, res


def kernel(**inputs):
    out, _ = run(inputs, trace=False)
    return out
